# revision 12
# baseline (speedup 1.0000x reference)
"""Causal self-attention (B=2, S=2048, E=2048, H=16) on 8 TRN2 NeuronCores.

Sharding: 2-way batch x 4-way head-group tensor parallel.
Core c handles batch c//4 and heads [4*(c%4), 4*(c%4)+4).

Per-core kernel (all matmuls single-product bf16, fp32 PSUM accumulation;
the 2e-2 correctness gate leaves plenty of headroom vs fp32):
  phase 1: X^T via PE transposes (bf16), interleaved with V projection
           (token-major: lhsT = X^T block, rhs = W_v) so PE work covers
           the X DMA stream
  phase 2: Q,K projections feature-major (lhsT = W block, rhs = X^T),
           SCALE folded into W_q/b_q on host, bias added during the
           psum->SBUF copy on the vector engine
  phase 3: per head: causal attention: q-major scores (bf16, chunked
           1024-wide psum), exp on scalar engine with accumulated row
           sums, in-place normalize (vector), P^T via PE transposes
           (bf16 psum), PV per 512-query group
  phase 4: out projection (bf16), partial over this core's 4 heads
Host: shard + bf16-cast inputs, SPMD on 8 cores, sum the 4 head-group
partials per batch and add (b_out + b_v @ W_out) once.
"""

from contextlib import ExitStack

import ml_dtypes
import numpy as np

import concourse.bass as bass
import concourse.tile as tile
from concourse import bacc, bass_utils, mybir
from concourse.masks import make_causal_mask, make_identity

FP = mybir.dt.float32
BF = mybir.dt.bfloat16
AF = mybir.ActivationFunctionType

B = 2
S = 2048
E = 2048
H = 16
HD = 128
NCORES = 8
HG = 4  # head-group axis (tensor parallel)
H_LOC = H // HG  # 4 heads per core
FLOC = H_LOC * HD  # 512 local features per q/k/v
SCALE = 1.0 / float(np.sqrt(HD))
NEG = -1.0e30

NB = S // 128  # 16 token blocks
EB = E // 128  # 16 contraction blocks

PROFILE = False
LAST_EXEC_NS = None
LAST_RESULTS = None


def _emit(nc):
    xh = nc.dram_tensor("xh", [S, E], BF, kind="ExternalInput").ap()
    wq_d = nc.dram_tensor("wq", [E, FLOC], BF, kind="ExternalInput").ap()
    wk_d = nc.dram_tensor("wk", [E, FLOC], BF, kind="ExternalInput").ap()
    wv_d = nc.dram_tensor("wv", [E, FLOC], BF, kind="ExternalInput").ap()
    bqk_d = nc.dram_tensor("bqk", [128, 2 * H_LOC], FP, kind="ExternalInput").ap()
    wo_d = nc.dram_tensor("wo", [FLOC, E], BF, kind="ExternalInput").ap()
    out = nc.dram_tensor("out", [S, E], FP, kind="ExternalOutput").ap()

    with tile.TileContext(nc) as tc, ExitStack() as top:
        cst = top.enter_context(tc.tile_pool(name="cst", bufs=1))
        ident_bf = cst.tile([128, 128], BF, name="identbf", tag="identbf")
        make_identity(nc, ident_bf[:])
        # cmaskT[k, q] = 0 where q >= k else NEG (transposed causal mask,
        # used as matmul rhs to mask k-major score blocks on the diagonal)
        cmaskT = cst.tile([128, 128], BF, name="cmaskT", tag="cmaskT")
        nc.gpsimd.memset(cmaskT[:], 0.0)
        nc.gpsimd.affine_select(
            out=cmaskT[:], in_=cmaskT[:],
            compare_op=mybir.AluOpType.is_ge, fill=NEG,
            base=0, pattern=[[1, 128]], channel_multiplier=-1,
        )
        ones_bf = cst.tile([128, 128], BF, name="ones", tag="ones")
        nc.gpsimd.memset(ones_bf[:], 1.0)
        bqk = cst.tile([128, 2 * H_LOC], FP, name="bqk", tag="bqk")
        nc.scalar.dma_start(bqk[:], bqk_d[:, :])

        # outputs of the projection phase, consumed by attention
        qkv_out = top.enter_context(tc.tile_pool(name="qkvo", bufs=1))
        qT = [qkv_out.tile([128, S], BF, name=f"qT{h}", tag=f"qT{h}")
              for h in range(H_LOC)]
        kT = [qkv_out.tile([128, S], BF, name=f"kT{h}", tag=f"kT{h}")
              for h in range(H_LOC)]
        # token-major V: for k-block j, head h: vsb[:, 512*j+128*h :][128, 128]
        vsb = qkv_out.tile([128, 4 * S], BF, name="vsb", tag="vsb")

        # PSUM pools: psF fp32 [128,1024] = 2 banks x 3 bufs = 6 banks,
        # psT bf16 [128,1024] = 1 bank x 2 bufs = 2 banks -> 8 total
        psF = top.enter_context(tc.tile_pool(name="psF", bufs=3, space="PSUM"))

        # ---------------- phase 1+2: X^T, V proj, Q/K proj ----------------
        with ExitStack() as ph, nc.named_scope("proj"):
            wpool = ph.enter_context(tc.tile_pool(name="w", bufs=1))
            wv_sb = [wpool.tile([128, FLOC], BF, name=f"wv{e}", tag=f"wv{e}")
                     for e in range(EB)]
            wq_sb = [wpool.tile([128, FLOC], BF, name=f"wq{e}", tag=f"wq{e}")
                     for e in range(EB)]
            wk_sb = [wpool.tile([128, FLOC], BF, name=f"wk{e}", tag=f"wk{e}")
                     for e in range(EB)]
            for e in range(EB):
                nc.scalar.dma_start(wv_sb[e][:], wv_d[128 * e : 128 * (e + 1), :])
            for e in range(EB):
                nc.scalar.dma_start(wq_sb[e][:], wq_d[128 * e : 128 * (e + 1), :])
                nc.scalar.dma_start(wk_sb[e][:], wk_d[128 * e : 128 * (e + 1), :])
            psT = ph.enter_context(tc.tile_pool(name="psT", bufs=2, space="PSUM"))
            xt_pool = ph.enter_context(tc.tile_pool(name="xt", bufs=1))
            xts = [xt_pool.tile([128, S], BF, name=f"xt{j}", tag=f"xt{j}")
                   for j in range(EB)]
            xin = ph.enter_context(tc.tile_pool(name="xin", bufs=8))

            def load_ig(ig):
                tiles = []
                for m in range(4):
                    i = 4 * ig + m
                    xr = xin.tile([128, E], BF, name="xin", tag="xin")
                    nc.sync.dma_start(xr[:], xh[128 * i : 128 * (i + 1), :])
                    tiles.append(xr)
                return tiles

            xrows = {0: load_ig(0), 1: load_ig(1)}
            for ig in range(NB // 4):
                xrow = xrows.pop(ig)
                if ig + 2 < NB // 4:
                    xrows[ig + 2] = load_ig(ig + 2)
                for j in range(EB):
                    pt = psT.tile([128, 1024], BF, name="pst", tag="pst")
                    for m in range(4):
                        nc.tensor.transpose(
                            pt[:, 128 * m : 128 * (m + 1)],
                            xrow[m][:, 128 * j : 128 * (j + 1)],
                            ident_bf[:],
                        )
                    nc.vector.tensor_copy(
                        xts[j][:, 512 * ig : 512 * (ig + 1)], pt[:, :512]
                    )
                # V projection for token blocks 4ig..4ig+3 (pairs)
                for m in range(0, 4, 2):
                    i = 4 * ig + m
                    ps = psF.tile([128, 1024], FP, name="psf", tag="psf")
                    for e in range(EB):
                        first = e == 0
                        last = e == EB - 1
                        nc.tensor.matmul(
                            ps[:, 0:512],
                            xts[e][:, 128 * i : 128 * (i + 1)],
                            wv_sb[e][:],
                            start=first, stop=last,
                        )
                        nc.tensor.matmul(
                            ps[:, 512:1024],
                            xts[e][:, 128 * (i + 1) : 128 * (i + 2)],
                            wv_sb[e][:],
                            start=first, stop=last,
                        )
                    nc.vector.tensor_copy(
                        vsb[:, 512 * i : 512 * (i + 2)], ps[:]
                    )

            # Q, K projections: feature-major [128 hd, S]
            for h in range(H_LOC):
                for which, wsb, dstl, bcol in (
                    (0, wq_sb, qT, h),
                    (1, wk_sb, kT, H_LOC + h),
                ):
                    for scp in range(S // 1024):
                        ps = psF.tile([128, 1024], FP, name="psf", tag="psf")
                        for e in range(EB):
                            wt = wsb[e][:, 128 * h : 128 * (h + 1)]
                            first = e == 0
                            last = e == EB - 1
                            nc.tensor.matmul(
                                ps[:, 0:512], wt,
                                xts[e][:, 1024 * scp : 1024 * scp + 512],
                                start=first, stop=last,
                            )
                            nc.tensor.matmul(
                                ps[:, 512:1024], wt,
                                xts[e][:, 1024 * scp + 512 : 1024 * (scp + 1)],
                                start=first, stop=last,
                            )
                        nc.vector.tensor_scalar_add(
                            dstl[h][:, 1024 * scp : 1024 * (scp + 1)],
                            ps[:], bqk[:, bcol : bcol + 1],
                        )

        # ---------------- phase 3: attention per head ----------------
        with ExitStack() as ao:
            att_pool = ao.enter_context(tc.tile_pool(name="att", bufs=1))
            attT = [att_pool.tile([128, S], BF, name=f"attT{h}", tag=f"attT{h}")
                    for h in range(H_LOC)]
            wo_pool = ao.enter_context(tc.tile_pool(name="wo", bufs=1))
            wo_sb = [wo_pool.tile([128, E], BF, name=f"wo{h}", tag=f"wo{h}")
                     for h in range(H_LOC)]
            for h in range(H_LOC):
                nc.scalar.dma_start(
                    wo_sb[h][:], wo_d[128 * h : 128 * (h + 1), :]
                )

            with ExitStack() as ph:
                pt_pool = ph.enter_context(tc.tile_pool(name="pt", bufs=4))
                rbg_pool = ph.enter_context(tc.tile_pool(name="rbg", bufs=4))
                PTs = {}

                from collections import deque

                pending = deque()

                def emit_some(n):
                    for _ in range(n):
                        if pending:
                            pending.popleft()()

                def sc_group(h, g, PT):
                    # k-major scores + exp for q-group g: PT slab j holds
                    # [128 k of block j, 512 q of group g] = exp(scores^T)
                    njj = 4 * g + 4
                    qgrp = qT[h][:, 512 * g : 512 * (g + 1)]
                    for jp in range(njj // 2):
                        ps = psF.tile([128, 1024], FP, name="psf", tag="psf")
                        for half in (0, 1):
                            jj = 2 * jp + half
                            base = 512 * half
                            m = jj - 4 * g
                            kblk = kT[h][:, 128 * jj : 128 * (jj + 1)]
                            if m < 0:
                                nc.tensor.matmul(
                                    ps[:, base : base + 512], kblk, qgrp,
                                    start=True, stop=True,
                                )
                            else:
                                q0 = 128 * m
                                nc.tensor.matmul(
                                    ps[:, base + q0 : base + q0 + 128], kblk,
                                    qT[h][:, 512 * g + q0 : 512 * g + q0 + 128],
                                    start=True, stop=False,
                                )
                                nc.tensor.matmul(
                                    ps[:, base + q0 : base + q0 + 128],
                                    ident_bf[:], cmaskT[:],
                                    start=False, stop=True,
                                )
                                if q0 + 128 < 512:
                                    nc.tensor.matmul(
                                        ps[:, base + q0 + 128 : base + 512],
                                        kblk,
                                        qT[h][:, 512 * g + q0 + 128 :
                                               512 * (g + 1)],
                                        start=True, stop=True,
                                    )
                        # exp the pair straight into PT (masked-out cols are
                        # uninitialized psum -> garbage, never read downstream)
                        nc.scalar.activation(
                            PT[:, 1024 * jp : 1024 * (jp + 1)], ps[:], AF.Exp,
                        )
                        if jp % 2 == 1:
                            emit_some(1)

                def sums_pv_pieces(h, g, PT):
                    njj = 4 * g + 4
                    shared = {}

                    def sums():
                        # po[:, 512:1024] = column sums of P, replicated to
                        # all 128 partitions by the all-ones stationary matrix
                        po = psF.tile([128, 1024], FP, name="psf", tag="psf")
                        shared["po"] = po
                        for jj in range(njj):
                            qlo = max(0, 128 * (jj - 4 * g))
                            nc.tensor.matmul(
                                po[:, 512 + qlo : 1024], ones_bf[:],
                                PT[:, 512 * jj + qlo : 512 * (jj + 1)],
                                start=(jj == 0), stop=(jj == njj - 1),
                            )

                    def pv():
                        po = shared["po"]
                        for jj in range(njj):
                            qlo = max(0, 128 * (jj - 4 * g))
                            nc.tensor.matmul(
                                po[:, qlo:512],
                                vsb[:, 512 * jj + 128 * h :
                                    512 * jj + 128 * (h + 1)],
                                PT[:, 512 * jj + qlo : 512 * (jj + 1)],
                                start=(jj == 0), stop=(jj == njj - 1),
                            )
                        # normalize while copying out of psum: attT slab =
                        # (P_unnorm @ V) * (1 / rowsums), exact by linearity
                        rbg = rbg_pool.tile([128, 512], FP, name="rbg",
                                            tag="rbg")
                        nc.vector.reciprocal(rbg[:], po[:, 512:1024])
                        nc.vector.tensor_mul(
                            attT[h][:, 512 * g : 512 * (g + 1)],
                            po[:, :512], rbg[:],
                        )
                    return [sums, pv]

                for hp in range(H_LOC // 2):
                    h0, h1 = 2 * hp, 2 * hp + 1
                    for g in range(4):
                        for h in (h0, h1):
                            with nc.named_scope(f"attn{h}"):
                                # fresh PT per (h, g): the ring WAR lands on
                                # a group finished long ago, not the previous
                                # group of this head
                                PT = pt_pool.tile([128, 4 * S], BF,
                                                  name="PT", tag="PT")
                                sc_group(h, g, PT)
                                pending.extend(sums_pv_pieces(h, g, PT))
                while pending:
                    pending.popleft()()

            # ---------------- phase 4: output projection ----------------
            with ExitStack() as ph, nc.named_scope("outproj"):
                ostg = ph.enter_context(tc.tile_pool(name="ostg", bufs=4))
                for i in range(NB):
                    psums = [psF.tile([128, 1024], FP, name="psf", tag="psf")
                             for _ in range(2)]
                    for h in range(H_LOC):
                        ah = attT[h][:, 128 * i : 128 * (i + 1)]
                        first = h == 0
                        last = h == H_LOC - 1
                        for c in range(4):
                            nc.tensor.matmul(
                                psums[c // 2][:, 512 * (c % 2) :
                                              512 * (c % 2 + 1)],
                                ah, wo_sb[h][:, 512 * c : 512 * (c + 1)],
                                start=first, stop=last,
                            )
                    for half in range(2):
                        ot = ostg.tile([128, 1024], FP, name="ostg", tag="ostg")
                        nc.vector.tensor_copy(ot[:], psums[half][:])
                        nc.sync.dma_start(
                            out[128 * i : 128 * (i + 1),
                                1024 * half : 1024 * (half + 1)],
                            ot[:],
                        )


_NC_CACHE = None


def _get_nc():
    global _NC_CACHE
    if _NC_CACHE is None:
        nc = bacc.Bacc(
            "TRN2",
            target_bir_lowering=False,
            debug=False,
            num_devices=1,
            enable_asserts=False,
        )
        _emit(nc)
        nc.compile()
        _NC_CACHE = nc
    return _NC_CACHE


def _bf(a):
    return np.ascontiguousarray(a.astype(ml_dtypes.bfloat16))


def make_in_maps(inX, W_qkv, b_qkv, W_out):
    xh = [_bf(inX[b]) for b in range(B)]
    per_hg = []
    for hg in range(HG):
        sl = slice(FLOC * hg, FLOC * (hg + 1))
        bqk = np.empty((128, 2 * H_LOC), dtype=np.float32)
        for h in range(H_LOC):
            f0 = FLOC * hg + 128 * h
            bqk[:, h] = b_qkv[f0 : f0 + 128] * SCALE
            bqk[:, H_LOC + h] = b_qkv[E + f0 : E + f0 + 128]
        per_hg.append(
            {
                "wq": _bf(W_qkv[:, 0:E][:, sl] * SCALE),
                "wk": _bf(W_qkv[:, E : 2 * E][:, sl]),
                "wv": _bf(W_qkv[:, 2 * E : 3 * E][:, sl]),
                "bqk": bqk,
                "wo": _bf(W_out[sl, :]),
            }
        )
    return [
        {"xh": xh[c // HG], **per_hg[c % HG]} for c in range(NCORES)
    ]


def kernel(inX, W_qkv, b_qkv, W_out, b_out):
    global LAST_EXEC_NS, LAST_RESULTS
    inX = np.asarray(inX, dtype=np.float32)
    W_qkv = np.asarray(W_qkv, dtype=np.float32)
    b_qkv = np.asarray(b_qkv, dtype=np.float32)
    W_out = np.asarray(W_out, dtype=np.float32)
    b_out = np.asarray(b_out, dtype=np.float32)

    nc = _get_nc()
    in_maps = make_in_maps(inX, W_qkv, b_qkv, W_out)

    kwargs = {}
    if PROFILE:
        kwargs = {"trace": True, "trace_cores": [0]}
    res = bass_utils.run_bass_kernel_spmd(
        nc, in_maps, core_ids=list(range(NCORES)), **kwargs
    )
    LAST_EXEC_NS = res.exec_time_ns
    LAST_RESULTS = res

    bias_full = (b_out + b_qkv[2 * E : 3 * E] @ W_out).astype(np.float32)
    out = np.empty((B, S, E), dtype=np.float32)
    for b in range(B):
        acc = res.results[HG * b + 0]["out"].astype(np.float64)
        for hg in range(1, HG):
            acc += res.results[HG * b + hg]["out"]
        out[b] = (acc + bias_full).astype(np.float32)
    return out


# revision 16
# speedup vs baseline: 1.0091x; 1.0091x over previous
"""Causal self-attention (B=2, S=2048, E=2048, H=16) on 8 TRN2 NeuronCores.

Sharding: 2-way batch x 4-way head-group tensor parallel.
Core c handles batch c//4 and heads [4*(c%4), 4*(c%4)+4).

Per-core kernel (all matmuls single-product bf16, fp32 PSUM accumulation;
the 2e-2 correctness gate leaves ample headroom vs fp32):
  phase 1: X^T via PE transposes (bf16 psum), prefetched X row DMAs,
           interleaved with V projection (token-major: lhsT = X^T block,
           rhs = W_v)
  phase 2: Q,K projections feature-major (lhsT = W block, rhs = X^T),
           SCALE folded into W_q/b_q on host, bias fused into the
           psum->SBUF copy on the vector engine
  phase 3: attention, two heads in lockstep, k-major: scores computed
           transposed (lhsT = K block, rhs = Q group) so exp writes the
           PV-ready layout directly (no P^T transposes); causal triangle
           zeroed post-exp on gpsimd; row sums via an all-ones stationary
           matrix (broadcast to all partitions); normalization deferred to
           the PV psum->SBUF copy (exact by linearity): attT = PV * recip
  phase 4: out projection (bf16), partial over this core's 4 heads
Host: shard + bf16-cast inputs, SPMD on 8 cores, sum the 4 head-group
partials per batch and add (b_out + b_v @ W_out) once.
"""

from collections import deque
from contextlib import ExitStack

import ml_dtypes
import numpy as np

import concourse.bass as bass
import concourse.tile as tile
from concourse import bacc, bass_utils, mybir
from concourse.masks import make_identity

FP = mybir.dt.float32
BF = mybir.dt.bfloat16
AF = mybir.ActivationFunctionType

B = 2
S = 2048
E = 2048
H = 16
HD = 128
NCORES = 8
HG = 4  # head-group axis (tensor parallel)
H_LOC = H // HG  # 4 heads per core
FLOC = H_LOC * HD  # 512 local features per q/k/v
SCALE = 1.0 / float(np.sqrt(HD))

NB = S // 128  # 16 token blocks
EB = E // 128  # 16 contraction blocks

PROFILE = False
LAST_EXEC_NS = None
LAST_RESULTS = None


def _emit(nc):
    xh = nc.dram_tensor("xh", [S, E], BF, kind="ExternalInput").ap()
    wq_d = nc.dram_tensor("wq", [E, FLOC], BF, kind="ExternalInput").ap()
    wk_d = nc.dram_tensor("wk", [E, FLOC], BF, kind="ExternalInput").ap()
    wv_d = nc.dram_tensor("wv", [E, FLOC], BF, kind="ExternalInput").ap()
    bqk_d = nc.dram_tensor("bqk", [128, 2 * H_LOC], FP, kind="ExternalInput").ap()
    wo_d = nc.dram_tensor("wo", [FLOC, E], BF, kind="ExternalInput").ap()
    out = nc.dram_tensor("out", [S, E], FP, kind="ExternalOutput").ap()

    with tile.TileContext(nc) as tc, ExitStack() as top:
        cst = top.enter_context(tc.tile_pool(name="cst", bufs=1))
        ident_bf = cst.tile([128, 128], BF, name="identbf", tag="identbf")
        make_identity(nc, ident_bf[:])
        ones_bf = cst.tile([128, 128], BF, name="ones", tag="ones")
        nc.gpsimd.memset(ones_bf[:], 1.0)
        bqk = cst.tile([128, 2 * H_LOC], FP, name="bqk", tag="bqk")
        nc.scalar.dma_start(bqk[:], bqk_d[:, :])

        # outputs of the projection phase, consumed by attention
        qkv_out = top.enter_context(tc.tile_pool(name="qkvo", bufs=1))
        qT = [qkv_out.tile([128, S], BF, name=f"qT{h}", tag=f"qT{h}")
              for h in range(H_LOC)]
        kT = [qkv_out.tile([128, S], BF, name=f"kT{h}", tag=f"kT{h}")
              for h in range(H_LOC)]
        # token-major V: for k-block j, head h: vsb[:, 512*j+128*h :][128, 128]
        vsb = qkv_out.tile([128, 4 * S], BF, name="vsb", tag="vsb")

        # ---------------- phase 1+2: X^T, V proj, Q/K proj ----------------
        # PSUM here: psP fp32 [128,512] x4 bufs = 4 banks + psT 2 banks
        with ExitStack() as ph, nc.named_scope("proj"):
            psT = ph.enter_context(tc.tile_pool(name="psT", bufs=2, space="PSUM"))
            psP = ph.enter_context(tc.tile_pool(name="psP", bufs=4, space="PSUM"))
            wpool = ph.enter_context(tc.tile_pool(name="w", bufs=1))
            wv_sb = [wpool.tile([128, FLOC], BF, name=f"wv{e}", tag=f"wv{e}")
                     for e in range(EB)]
            wq_sb = [wpool.tile([128, FLOC], BF, name=f"wq{e}", tag=f"wq{e}")
                     for e in range(EB)]
            wk_sb = [wpool.tile([128, FLOC], BF, name=f"wk{e}", tag=f"wk{e}")
                     for e in range(EB)]
            for e in range(EB):
                nc.scalar.dma_start(wv_sb[e][:], wv_d[128 * e : 128 * (e + 1), :])
            for e in range(EB):
                nc.scalar.dma_start(wq_sb[e][:], wq_d[128 * e : 128 * (e + 1), :])
                nc.scalar.dma_start(wk_sb[e][:], wk_d[128 * e : 128 * (e + 1), :])
            xt_pool = ph.enter_context(tc.tile_pool(name="xt", bufs=1))
            xts = [xt_pool.tile([128, S], BF, name=f"xt{j}", tag=f"xt{j}")
                   for j in range(EB)]
            xin = ph.enter_context(tc.tile_pool(name="xin", bufs=8))

            def load_ig(ig):
                tiles = []
                for m in range(4):
                    i = 4 * ig + m
                    xr = xin.tile([128, E], BF, name="xin", tag="xin")
                    eng = nc.scalar if (ig == 0 and m % 2) else nc.sync
                    eng.dma_start(xr[:], xh[128 * i : 128 * (i + 1), :])
                    tiles.append(xr)
                return tiles

            xrows = {0: load_ig(0), 1: load_ig(1)}
            for ig in range(NB // 4):
                xrow = xrows.pop(ig)
                if ig + 2 < NB // 4:
                    xrows[ig + 2] = load_ig(ig + 2)
                for j in range(EB):
                    pt = psT.tile([128, 1024], BF, name="pst", tag="pst")
                    for m in range(4):
                        nc.tensor.transpose(
                            pt[:, 128 * m : 128 * (m + 1)],
                            xrow[m][:, 128 * j : 128 * (j + 1)],
                            ident_bf[:],
                        )
                    nc.vector.tensor_copy(
                        xts[j][:, 512 * ig : 512 * (ig + 1)], pt[:, :512]
                    )
                # V projection for the 4 token blocks of this group
                for m in range(4):
                    i = 4 * ig + m
                    ps = psP.tile([128, 512], FP, name="psp", tag="psp")
                    for e in range(EB):
                        nc.tensor.matmul(
                            ps[:],
                            xts[e][:, 128 * i : 128 * (i + 1)],
                            wv_sb[e][:],
                            start=(e == 0), stop=(e == EB - 1),
                        )
                    nc.vector.tensor_copy(vsb[:, 512 * i : 512 * (i + 1)], ps[:])

            # Q, K projections: feature-major [128 hd, S]
            for h in range(H_LOC):
                for wsb, dstl, bcol in (
                    (wq_sb, qT, h),
                    (wk_sb, kT, H_LOC + h),
                ):
                    for sc in range(4):
                        ps = psP.tile([128, 512], FP, name="psp", tag="psp")
                        for e in range(EB):
                            nc.tensor.matmul(
                                ps[:],
                                wsb[e][:, 128 * h : 128 * (h + 1)],
                                xts[e][:, 512 * sc : 512 * (sc + 1)],
                                start=(e == 0), stop=(e == EB - 1),
                            )
                        nc.vector.tensor_scalar_add(
                            dstl[h][:, 512 * sc : 512 * (sc + 1)],
                            ps[:], bqk[:, bcol : bcol + 1],
                        )

        # ---------------- phase 3+4: attention + out projection ----------
        with ExitStack() as ao:
            att_pool = ao.enter_context(tc.tile_pool(name="att", bufs=1))
            attT = [att_pool.tile([128, S], BF, name=f"attT{h}", tag=f"attT{h}")
                    for h in range(H_LOC)]
            wo_pool = ao.enter_context(tc.tile_pool(name="wo", bufs=1))
            wo_sb = [wo_pool.tile([128, E], BF, name=f"wo{h}", tag=f"wo{h}")
                     for h in range(H_LOC)]
            for h in range(H_LOC):
                nc.scalar.dma_start(wo_sb[h][:], wo_d[128 * h : 128 * (h + 1), :])
            # psF fp32 [128,512]: PV/rowsum pieces, then outproj chains
            psF = ao.enter_context(tc.tile_pool(name="psF", bufs=2, space="PSUM"))

            with ExitStack() as ph:
                pt_pool = ph.enter_context(tc.tile_pool(name="pt", bufs=4))
                rbg_pool = ph.enter_context(tc.tile_pool(name="rbg", bufs=4))
                # scores psum: fp32 [128,1024] = 2 banks x3 -> 6 banks
                psSC = ph.enter_context(
                    tc.tile_pool(name="psSC", bufs=3, space="PSUM")
                )

                pending = deque()

                def emit_some(n):
                    for _ in range(n):
                        if pending:
                            pending.popleft()()

                def sc_group(h, g, PT):
                    # k-major scores + exp for q-group g: PT slab j holds
                    # [128 k of block j, 512 q of group g] = exp(scores^T).
                    # One full-width matmul per block; causal triangle zeroed
                    # post-exp on gpsimd; fully-below-diagonal cols of diag
                    # slabs are garbage, skipped via qlo windows in sums/PV.
                    njj = 4 * g + 4
                    qgrp = qT[h][:, 512 * g : 512 * (g + 1)]
                    for jp in range(njj // 2):
                        # pop filler BEFORE the ring allocation: the PE queue
                        # then holds independent work ahead of any alloc stall
                        emit_some(1)
                        ps = psSC.tile([128, 1024], FP, name="pssc", tag="pssc")
                        for half in (0, 1):
                            jj = 2 * jp + half
                            nc.tensor.matmul(
                                ps[:, 512 * half : 512 * (half + 1)],
                                kT[h][:, 128 * jj : 128 * (jj + 1)], qgrp,
                                start=True, stop=True,
                            )
                        nc.scalar.activation(
                            PT[:, 1024 * jp : 1024 * (jp + 1)], ps[:], AF.Exp,
                        )
                        for half in (0, 1):
                            jj = 2 * jp + half
                            m = jj - 4 * g
                            if 0 <= m < 4:
                                # zero P where q < k inside the diagonal block
                                blk = PT[:, 512 * jj + 128 * m :
                                         512 * jj + 128 * (m + 1)]
                                nc.gpsimd.affine_select(
                                    out=blk, in_=blk,
                                    compare_op=mybir.AluOpType.is_ge,
                                    fill=0.0, base=0,
                                    pattern=[[1, 128]], channel_multiplier=-1,
                                )

                def sums_pv_pieces(h, g, PT):
                    njj = 4 * g + 4
                    shared = {}

                    def sums():
                        # rowsums replicated to all partitions by the all-ones
                        # stationary matrix
                        po = psF.tile([128, 512], FP, name="psf", tag="psf")
                        shared["sums"] = po
                        for jj in range(njj):
                            qlo = max(0, 128 * (jj - 4 * g))
                            nc.tensor.matmul(
                                po[:, qlo:512], ones_bf[:],
                                PT[:, 512 * jj + qlo : 512 * (jj + 1)],
                                start=(jj == 0), stop=(jj == njj - 1),
                            )

                    def pv():
                        po = psF.tile([128, 512], FP, name="psf", tag="psf")
                        for jj in range(njj):
                            qlo = max(0, 128 * (jj - 4 * g))
                            nc.tensor.matmul(
                                po[:, qlo:512],
                                vsb[:, 512 * jj + 128 * h :
                                    512 * jj + 128 * (h + 1)],
                                PT[:, 512 * jj + qlo : 512 * (jj + 1)],
                                start=(jj == 0), stop=(jj == njj - 1),
                            )
                        # normalize while copying out of psum: attT slab =
                        # (P_unnorm @ V) * (1 / rowsums), exact by linearity
                        rbg = rbg_pool.tile([128, 512], FP, name="rbg",
                                            tag="rbg")
                        nc.vector.reciprocal(rbg[:], shared["sums"][:])
                        nc.vector.tensor_mul(
                            attT[h][:, 512 * g : 512 * (g + 1)], po[:], rbg[:]
                        )
                    return [sums, pv]

                for hp in range(H_LOC // 2):
                    h0, h1 = 2 * hp, 2 * hp + 1
                    for g in range(4):
                        for h in (h0, h1):
                            with nc.named_scope(f"attn{h}"):
                                PT = pt_pool.tile([128, 4 * S], BF,
                                                  name="PT", tag="PT")
                                sc_group(h, g, PT)
                                pending.extend(sums_pv_pieces(h, g, PT))
                while pending:
                    pending.popleft()()

            # ---------------- phase 4: output projection ----------------
            with ExitStack() as ph, nc.named_scope("outproj"):
                ostg = ph.enter_context(tc.tile_pool(name="ostg", bufs=4))
                for i in range(NB):
                    for cp in range(2):
                        pss = [psF.tile([128, 512], FP, name="psf", tag="psf")
                               for _ in range(2)]
                        for h in range(H_LOC):
                            ah = attT[h][:, 128 * i : 128 * (i + 1)]
                            for k in range(2):
                                c = 2 * cp + k
                                nc.tensor.matmul(
                                    pss[k][:], ah,
                                    wo_sb[h][:, 512 * c : 512 * (c + 1)],
                                    start=(h == 0), stop=(h == H_LOC - 1),
                                )
                        for k in range(2):
                            c = 2 * cp + k
                            ot = ostg.tile([128, 512], FP, name="ostg",
                                           tag="ostg")
                            nc.vector.tensor_copy(ot[:], pss[k][:])
                            nc.sync.dma_start(
                                out[128 * i : 128 * (i + 1),
                                    512 * c : 512 * (c + 1)],
                                ot[:],
                            )


_NC_CACHE = None


def _get_nc():
    global _NC_CACHE
    if _NC_CACHE is None:
        nc = bacc.Bacc(
            "TRN2",
            target_bir_lowering=False,
            debug=False,
            num_devices=1,
            enable_asserts=False,
        )
        _emit(nc)
        nc.compile()
        _NC_CACHE = nc
    return _NC_CACHE


def _bf(a):
    return np.ascontiguousarray(a.astype(ml_dtypes.bfloat16))


def make_in_maps(inX, W_qkv, b_qkv, W_out):
    xh = [_bf(inX[b]) for b in range(B)]
    per_hg = []
    for hg in range(HG):
        sl = slice(FLOC * hg, FLOC * (hg + 1))
        bqk = np.empty((128, 2 * H_LOC), dtype=np.float32)
        for h in range(H_LOC):
            f0 = FLOC * hg + 128 * h
            bqk[:, h] = b_qkv[f0 : f0 + 128] * SCALE
            bqk[:, H_LOC + h] = b_qkv[E + f0 : E + f0 + 128]
        per_hg.append(
            {
                "wq": _bf(W_qkv[:, 0:E][:, sl] * SCALE),
                "wk": _bf(W_qkv[:, E : 2 * E][:, sl]),
                "wv": _bf(W_qkv[:, 2 * E : 3 * E][:, sl]),
                "bqk": bqk,
                "wo": _bf(W_out[sl, :]),
            }
        )
    return [
        {"xh": xh[c // HG], **per_hg[c % HG]} for c in range(NCORES)
    ]


def kernel(inX, W_qkv, b_qkv, W_out, b_out):
    global LAST_EXEC_NS, LAST_RESULTS
    inX = np.asarray(inX, dtype=np.float32)
    W_qkv = np.asarray(W_qkv, dtype=np.float32)
    b_qkv = np.asarray(b_qkv, dtype=np.float32)
    W_out = np.asarray(W_out, dtype=np.float32)
    b_out = np.asarray(b_out, dtype=np.float32)

    nc = _get_nc()
    in_maps = make_in_maps(inX, W_qkv, b_qkv, W_out)

    kwargs = {}
    if PROFILE:
        kwargs = {"trace": True, "trace_cores": [0]}
    res = bass_utils.run_bass_kernel_spmd(
        nc, in_maps, core_ids=list(range(NCORES)), **kwargs
    )
    LAST_EXEC_NS = res.exec_time_ns
    LAST_RESULTS = res

    bias_full = (b_out + b_qkv[2 * E : 3 * E] @ W_out).astype(np.float32)
    out = np.empty((B, S, E), dtype=np.float32)
    for b in range(B):
        acc = res.results[HG * b + 0]["out"].astype(np.float64)
        for hg in range(1, HG):
            acc += res.results[HG * b + hg]["out"]
        out[b] = (acc + bias_full).astype(np.float32)
    return out


# revision 17
# speedup vs baseline: 1.1441x; 1.1338x over previous
"""Round-3 kernel (best proven: 395,148 ns): q-major lockstep attention."""

from collections import deque
from contextlib import ExitStack

import ml_dtypes
import numpy as np

import concourse.bass as bass
import concourse.tile as tile
from concourse import bacc, bass_utils, mybir
from concourse.masks import make_causal_mask, make_identity

FP = mybir.dt.float32
BF = mybir.dt.bfloat16
AF = mybir.ActivationFunctionType

B = 2
S = 2048
E = 2048
H = 16
HD = 128
NCORES = 8
HG = 4
H_LOC = H // HG
FLOC = H_LOC * HD
SCALE = 1.0 / float(np.sqrt(HD))
NEG = -1.0e30

NB = S // 128
EB = E // 128

PROFILE = False
LAST_EXEC_NS = None
LAST_RESULTS = None


def _emit(nc):
    xh = nc.dram_tensor("xh", [S, E], BF, kind="ExternalInput").ap()
    wq_d = nc.dram_tensor("wq", [E, FLOC], BF, kind="ExternalInput").ap()
    wk_d = nc.dram_tensor("wk", [E, FLOC], BF, kind="ExternalInput").ap()
    wv_d = nc.dram_tensor("wv", [E, FLOC], BF, kind="ExternalInput").ap()
    bqk_d = nc.dram_tensor("bqk", [128, 2 * H_LOC], FP, kind="ExternalInput").ap()
    wo_d = nc.dram_tensor("wo", [FLOC, E], BF, kind="ExternalInput").ap()
    out = nc.dram_tensor("out", [S, E], FP, kind="ExternalOutput").ap()

    with tile.TileContext(nc) as tc, ExitStack() as top:
        cst = top.enter_context(tc.tile_pool(name="cst", bufs=1))
        ident_bf = cst.tile([128, 128], BF, name="identbf", tag="identbf")
        make_identity(nc, ident_bf[:])
        cmask = cst.tile([128, 128], BF, name="cmask", tag="cmask")
        make_causal_mask(nc, cmask[:], mask_val=NEG)
        bqk = cst.tile([128, 2 * H_LOC], FP, name="bqk", tag="bqk")
        nc.scalar.dma_start(bqk[:], bqk_d[:, :])

        qkv_out = top.enter_context(tc.tile_pool(name="qkvo", bufs=1))
        qT = [qkv_out.tile([128, S], BF, name=f"qT{h}", tag=f"qT{h}")
              for h in range(H_LOC)]
        kT = [qkv_out.tile([128, S], BF, name=f"kT{h}", tag=f"kT{h}")
              for h in range(H_LOC)]
        vsb = qkv_out.tile([128, 4 * S], BF, name="vsb", tag="vsb")

        psF = top.enter_context(tc.tile_pool(name="psF", bufs=3, space="PSUM"))
        psT = top.enter_context(tc.tile_pool(name="psT", bufs=2, space="PSUM"))

        with ExitStack() as ph, nc.named_scope("proj"):
            wpool = ph.enter_context(tc.tile_pool(name="w", bufs=1))
            wv_sb = [wpool.tile([128, FLOC], BF, name=f"wv{e}", tag=f"wv{e}")
                     for e in range(EB)]
            wq_sb = [wpool.tile([128, FLOC], BF, name=f"wq{e}", tag=f"wq{e}")
                     for e in range(EB)]
            wk_sb = [wpool.tile([128, FLOC], BF, name=f"wk{e}", tag=f"wk{e}")
                     for e in range(EB)]
            for e in range(EB):
                nc.scalar.dma_start(wv_sb[e][:], wv_d[128 * e : 128 * (e + 1), :])
            for e in range(EB):
                nc.scalar.dma_start(wq_sb[e][:], wq_d[128 * e : 128 * (e + 1), :])
                nc.scalar.dma_start(wk_sb[e][:], wk_d[128 * e : 128 * (e + 1), :])
            xt_pool = ph.enter_context(tc.tile_pool(name="xt", bufs=1))
            xts = [xt_pool.tile([128, S], BF, name=f"xt{j}", tag=f"xt{j}")
                   for j in range(EB)]
            xin = ph.enter_context(tc.tile_pool(name="xin", bufs=8))

            def load_ig(ig):
                tiles = []
                for m in range(4):
                    i = 4 * ig + m
                    xr = xin.tile([128, E], BF, name="xin", tag="xin")
                    nc.sync.dma_start(xr[:], xh[128 * i : 128 * (i + 1), :])
                    tiles.append(xr)
                return tiles

            xrows = {0: load_ig(0), 1: load_ig(1)}
            for ig in range(NB // 4):
                xrow = xrows.pop(ig)
                if ig + 2 < NB // 4:
                    xrows[ig + 2] = load_ig(ig + 2)
                for j in range(EB):
                    pt = psT.tile([128, 1024], BF, name="pst", tag="pst")
                    for m in range(4):
                        nc.tensor.transpose(
                            pt[:, 128 * m : 128 * (m + 1)],
                            xrow[m][:, 128 * j : 128 * (j + 1)],
                            ident_bf[:],
                        )
                    nc.vector.tensor_copy(
                        xts[j][:, 512 * ig : 512 * (ig + 1)], pt[:, :512]
                    )
                for m in range(0, 4, 2):
                    i = 4 * ig + m
                    ps = psF.tile([128, 1024], FP, name="psf", tag="psf")
                    for e in range(EB):
                        first = e == 0
                        last = e == EB - 1
                        nc.tensor.matmul(
                            ps[:, 0:512],
                            xts[e][:, 128 * i : 128 * (i + 1)],
                            wv_sb[e][:],
                            start=first, stop=last,
                        )
                        nc.tensor.matmul(
                            ps[:, 512:1024],
                            xts[e][:, 128 * (i + 1) : 128 * (i + 2)],
                            wv_sb[e][:],
                            start=first, stop=last,
                        )
                    nc.vector.tensor_copy(
                        vsb[:, 512 * i : 512 * (i + 2)], ps[:]
                    )

            for h in range(H_LOC):
                for which, wsb, dstl, bcol in (
                    (0, wq_sb, qT, h),
                    (1, wk_sb, kT, H_LOC + h),
                ):
                    for scp in range(S // 1024):
                        ps = psF.tile([128, 1024], FP, name="psf", tag="psf")
                        for e in range(EB):
                            wt = wsb[e][:, 128 * h : 128 * (h + 1)]
                            first = e == 0
                            last = e == EB - 1
                            nc.tensor.matmul(
                                ps[:, 0:512], wt,
                                xts[e][:, 1024 * scp : 1024 * scp + 512],
                                start=first, stop=last,
                            )
                            nc.tensor.matmul(
                                ps[:, 512:1024], wt,
                                xts[e][:, 1024 * scp + 512 : 1024 * (scp + 1)],
                                start=first, stop=last,
                            )
                        nc.vector.tensor_scalar_add(
                            dstl[h][:, 1024 * scp : 1024 * (scp + 1)],
                            ps[:], bqk[:, bcol : bcol + 1],
                        )

        with ExitStack() as ao:
            att_pool = ao.enter_context(tc.tile_pool(name="att", bufs=1))
            attT = [att_pool.tile([128, S], BF, name=f"attT{h}", tag=f"attT{h}")
                    for h in range(H_LOC)]
            wo_pool = ao.enter_context(tc.tile_pool(name="wo", bufs=1))
            wo_sb = [wo_pool.tile([128, E], BF, name=f"wo{h}", tag=f"wo{h}")
                     for h in range(H_LOC)]
            for h in range(H_LOC):
                nc.scalar.dma_start(
                    wo_sb[h][:], wo_d[128 * h : 128 * (h + 1), :]
                )

            with ExitStack() as ph:
                p_pool = ph.enter_context(tc.tile_pool(name="p", bufs=14))
                pt_pool = ph.enter_context(tc.tile_pool(name="pt", bufs=2))
                rs_pool = ph.enter_context(tc.tile_pool(name="rs", bufs=10))
                PTs = {}

                pending = deque()

                def emit_some(n):
                    for _ in range(n):
                        if pending:
                            pending.popleft()()

                def sc_block(h, g, qs, pts):
                    i = 4 * g + qs
                    L = 128 * (i + 1)
                    nch = (L + 1023) // 1024
                    p_i = p_pool.tile([128, S], BF, name="p", tag="p")
                    rs = rs_pool.tile([128, 4], FP, name="rs", tag="rs")
                    pts.append(p_i)
                    qblk = qT[h][:, 128 * i : 128 * (i + 1)]
                    for c in range(nch):
                        w = min(1024, L - 1024 * c)
                        base = 1024 * c
                        ps = psF.tile([128, 1024], FP, name="psf", tag="psf")
                        if c == nch - 1:
                            pre = w - 128
                            s = 0
                            while s < pre:
                                e2 = min(s + 512, pre)
                                nc.tensor.matmul(
                                    ps[:, s:e2], qblk,
                                    kT[h][:, base + s : base + e2],
                                    start=True, stop=True,
                                )
                                s = e2
                            nc.tensor.matmul(
                                ps[:, pre:w], qblk,
                                kT[h][:, base + pre : base + w],
                                start=True, stop=False,
                            )
                            nc.tensor.matmul(
                                ps[:, pre:w], ident_bf[:], cmask[:],
                                start=False, stop=True,
                            )
                        else:
                            for hf in (0, 512):
                                nc.tensor.matmul(
                                    ps[:, hf : hf + 512], qblk,
                                    kT[h][:, base + hf : base + hf + 512],
                                    start=True, stop=True,
                                )
                        nc.scalar.activation(
                            p_i[:, base : base + w], ps[:, :w], AF.Exp,
                            accum_out=rs[:, c : c + 1],
                        )
                    if nch > 1:
                        nc.vector.tensor_add(rs[:, 0:1], rs[:, 0:1], rs[:, 1:2])
                    nc.vector.reciprocal(rs[:, 2:3], rs[:, 0:1])
                    nc.vector.tensor_scalar_mul(p_i[:, :L], p_i[:, :L], rs[:, 2:3])

                def pt_pv_pieces(h, g, pts):
                    PT = PTs[h]
                    pieces = []
                    for jp in range(2 * g + 2):
                        def tr(jp=jp):
                            pt = psT.tile([128, 1024], BF, name="pst", tag="pst")
                            for jj, base in ((2 * jp, 0), (2 * jp + 1, 512)):
                                for qs in range(max(0, jj - 4 * g), 4):
                                    nc.tensor.transpose(
                                        pt[:, base + 128 * qs : base + 128 * (qs + 1)],
                                        pts[qs][:, 128 * jj : 128 * (jj + 1)],
                                        ident_bf[:],
                                    )
                            if jp < 2 * g:
                                nc.vector.tensor_copy(
                                    PT[:, 1024 * jp : 1024 * (jp + 1)], pt[:]
                                )
                            else:
                                m0 = 2 * jp - 4 * g
                                nc.vector.tensor_copy(
                                    PT[:, 1024 * jp + 128 * m0 : 1024 * jp + 512],
                                    pt[:, 128 * m0 : 512],
                                )
                                nc.vector.tensor_copy(
                                    PT[:, 1024 * jp + 512 + 128 * (m0 + 1) :
                                       1024 * (jp + 1)],
                                    pt[:, 512 + 128 * (m0 + 1) : 1024],
                                )
                        pieces.append(tr)

                    def pv():
                        po = psF.tile([128, 1024], FP, name="psf", tag="psf")
                        nkc = 4 * g + 4
                        for j in range(nkc):
                            qlo = max(0, 128 * (j - 4 * g))
                            nc.tensor.matmul(
                                po[:, qlo:512],
                                vsb[:, 512 * j + 128 * h : 512 * j + 128 * (h + 1)],
                                PT[:, 512 * j + qlo : 512 * (j + 1)],
                                start=(j == 0), stop=(j == nkc - 1),
                            )
                        nc.vector.tensor_copy(
                            attT[h][:, 512 * g : 512 * (g + 1)], po[:, :512]
                        )
                    pieces.append(pv)
                    return pieces

                for hp in range(H_LOC // 2):
                    h0, h1 = 2 * hp, 2 * hp + 1
                    PTs[h0] = pt_pool.tile([128, 4 * S], BF, name="PT", tag="PT")
                    PTs[h1] = pt_pool.tile([128, 4 * S], BF, name="PT", tag="PT")
                    for g in range(4):
                        for h in (h0, h1):
                            with nc.named_scope(f"attn{h}"):
                                pts = []
                                for qs in range(4):
                                    sc_block(h, g, qs, pts)
                                    emit_some(2 if g >= 2 else 1)
                                pending.extend(pt_pv_pieces(h, g, pts))
                    while pending:
                        pending.popleft()()

            with ExitStack() as ph, nc.named_scope("outproj"):
                ostg = ph.enter_context(tc.tile_pool(name="ostg", bufs=4))
                for i in range(NB):
                    psums = [psF.tile([128, 1024], FP, name="psf", tag="psf")
                             for _ in range(2)]
                    for h in range(H_LOC):
                        ah = attT[h][:, 128 * i : 128 * (i + 1)]
                        first = h == 0
                        last = h == H_LOC - 1
                        for c in range(4):
                            nc.tensor.matmul(
                                psums[c // 2][:, 512 * (c % 2) : 512 * (c % 2 + 1)],
                                ah, wo_sb[h][:, 512 * c : 512 * (c + 1)],
                                start=first, stop=last,
                            )
                    for half in range(2):
                        ot = ostg.tile([128, 1024], FP, name="ostg", tag="ostg")
                        nc.vector.tensor_copy(ot[:], psums[half][:])
                        nc.sync.dma_start(
                            out[128 * i : 128 * (i + 1),
                                1024 * half : 1024 * (half + 1)],
                            ot[:],
                        )


_NC_CACHE = None


def _get_nc():
    global _NC_CACHE
    if _NC_CACHE is None:
        nc = bacc.Bacc(
            "TRN2",
            target_bir_lowering=False,
            debug=False,
            num_devices=1,
            enable_asserts=False,
        )
        _emit(nc)
        nc.compile()
        _NC_CACHE = nc
    return _NC_CACHE


def _bf(a):
    return np.ascontiguousarray(a.astype(ml_dtypes.bfloat16))


def make_in_maps(inX, W_qkv, b_qkv, W_out):
    xh = [_bf(inX[b]) for b in range(B)]
    per_hg = []
    for hg in range(HG):
        sl = slice(FLOC * hg, FLOC * (hg + 1))
        bqk = np.empty((128, 2 * H_LOC), dtype=np.float32)
        for h in range(H_LOC):
            f0 = FLOC * hg + 128 * h
            bqk[:, h] = b_qkv[f0 : f0 + 128] * SCALE
            bqk[:, H_LOC + h] = b_qkv[E + f0 : E + f0 + 128]
        per_hg.append(
            {
                "wq": _bf(W_qkv[:, 0:E][:, sl] * SCALE),
                "wk": _bf(W_qkv[:, E : 2 * E][:, sl]),
                "wv": _bf(W_qkv[:, 2 * E : 3 * E][:, sl]),
                "bqk": bqk,
                "wo": _bf(W_out[sl, :]),
            }
        )
    return [
        {"xh": xh[c // HG], **per_hg[c % HG]} for c in range(NCORES)
    ]


def kernel(inX, W_qkv, b_qkv, W_out, b_out):
    global LAST_EXEC_NS, LAST_RESULTS
    inX = np.asarray(inX, dtype=np.float32)
    W_qkv = np.asarray(W_qkv, dtype=np.float32)
    b_qkv = np.asarray(b_qkv, dtype=np.float32)
    W_out = np.asarray(W_out, dtype=np.float32)
    b_out = np.asarray(b_out, dtype=np.float32)

    nc = _get_nc()
    in_maps = make_in_maps(inX, W_qkv, b_qkv, W_out)

    kwargs = {}
    if PROFILE:
        kwargs = {"trace": True, "trace_cores": [0]}
    res = bass_utils.run_bass_kernel_spmd(
        nc, in_maps, core_ids=list(range(NCORES)), **kwargs
    )
    LAST_EXEC_NS = res.exec_time_ns
    LAST_RESULTS = res

    bias_full = (b_out + b_qkv[2 * E : 3 * E] @ W_out).astype(np.float32)
    out = np.empty((B, S, E), dtype=np.float32)
    for b in range(B):
        acc = res.results[HG * b + 0]["out"].astype(np.float64)
        for hg in range(1, HG):
            acc += res.results[HG * b + hg]["out"]
        out[b] = (acc + bias_full).astype(np.float32)
    return out


# revision 18
# speedup vs baseline: 1.1479x; 1.0033x over previous
"""Round-10: r3 q-major attention + r9 proj (psP chains) + early X DMA."""

from collections import deque
from contextlib import ExitStack

import ml_dtypes
import numpy as np

import concourse.bass as bass
import concourse.tile as tile
from concourse import bacc, bass_utils, mybir
from concourse.masks import make_causal_mask, make_identity

FP = mybir.dt.float32
BF = mybir.dt.bfloat16
AF = mybir.ActivationFunctionType

B = 2
S = 2048
E = 2048
H = 16
HD = 128
NCORES = 8
HG = 4
H_LOC = H // HG
FLOC = H_LOC * HD
SCALE = 1.0 / float(np.sqrt(HD))
NEG = -1.0e30

NB = S // 128
EB = E // 128

PROFILE = False
LAST_EXEC_NS = None
LAST_RESULTS = None


def _emit(nc):
    xh = nc.dram_tensor("xh", [S, E], BF, kind="ExternalInput").ap()
    wq_d = nc.dram_tensor("wq", [E, FLOC], BF, kind="ExternalInput").ap()
    wk_d = nc.dram_tensor("wk", [E, FLOC], BF, kind="ExternalInput").ap()
    wv_d = nc.dram_tensor("wv", [E, FLOC], BF, kind="ExternalInput").ap()
    bqk_d = nc.dram_tensor("bqk", [128, 2 * H_LOC], FP, kind="ExternalInput").ap()
    wo_d = nc.dram_tensor("wo", [FLOC, E], BF, kind="ExternalInput").ap()
    out = nc.dram_tensor("out", [S, E], FP, kind="ExternalOutput").ap()

    with tile.TileContext(nc) as tc, ExitStack() as top:
        cst = top.enter_context(tc.tile_pool(name="cst", bufs=1))
        ident_bf = cst.tile([128, 128], BF, name="identbf", tag="identbf")
        make_identity(nc, ident_bf[:])
        cmask = cst.tile([128, 128], BF, name="cmask", tag="cmask")
        make_causal_mask(nc, cmask[:], mask_val=NEG)
        bqk = cst.tile([128, 2 * H_LOC], FP, name="bqk", tag="bqk")
        nc.scalar.dma_start(bqk[:], bqk_d[:, :])

        qkv_out = top.enter_context(tc.tile_pool(name="qkvo", bufs=1))
        qT = [qkv_out.tile([128, S], BF, name=f"qT{h}", tag=f"qT{h}")
              for h in range(H_LOC)]
        kT = [qkv_out.tile([128, S], BF, name=f"kT{h}", tag=f"kT{h}")
              for h in range(H_LOC)]
        vsb = qkv_out.tile([128, 4 * S], BF, name="vsb", tag="vsb")

        with ExitStack() as ph, nc.named_scope("proj"):
            psT = ph.enter_context(tc.tile_pool(name="psT", bufs=2, space="PSUM"))
            psP = ph.enter_context(tc.tile_pool(name="psP", bufs=4, space="PSUM"))
            wpool = ph.enter_context(tc.tile_pool(name="w", bufs=1))
            wv_sb = [wpool.tile([128, FLOC], BF, name=f"wv{e}", tag=f"wv{e}")
                     for e in range(EB)]
            wq_sb = [wpool.tile([128, FLOC], BF, name=f"wq{e}", tag=f"wq{e}")
                     for e in range(EB)]
            wk_sb = [wpool.tile([128, FLOC], BF, name=f"wk{e}", tag=f"wk{e}")
                     for e in range(EB)]
            xt_pool = ph.enter_context(tc.tile_pool(name="xt", bufs=1))
            xts = [xt_pool.tile([128, S], BF, name=f"xt{j}", tag=f"xt{j}")
                   for j in range(EB)]
            xin = ph.enter_context(tc.tile_pool(name="xin", bufs=8))

            def load_ig(ig):
                tiles = []
                for m in range(4):
                    i = 4 * ig + m
                    xr = xin.tile([128, E], BF, name="xin", tag="xin")
                    # first group: split across both HWDGE rings, ahead of
                    # the weight stream, to cut the cold-start latency
                    eng = nc.scalar if (ig == 0 and m % 2) else nc.sync
                    eng.dma_start(xr[:], xh[128 * i : 128 * (i + 1), :])
                    tiles.append(xr)
                return tiles

            xrows = {0: load_ig(0), 1: load_ig(1)}
            for e in range(EB):
                nc.scalar.dma_start(wv_sb[e][:], wv_d[128 * e : 128 * (e + 1), :])
            for e in range(EB):
                nc.scalar.dma_start(wq_sb[e][:], wq_d[128 * e : 128 * (e + 1), :])
                nc.scalar.dma_start(wk_sb[e][:], wk_d[128 * e : 128 * (e + 1), :])
            for ig in range(NB // 4):
                xrow = xrows.pop(ig)
                if ig + 2 < NB // 4:
                    xrows[ig + 2] = load_ig(ig + 2)
                for j in range(EB):
                    pt = psT.tile([128, 1024], BF, name="pst", tag="pst")
                    for m in range(4):
                        nc.tensor.transpose(
                            pt[:, 128 * m : 128 * (m + 1)],
                            xrow[m][:, 128 * j : 128 * (j + 1)],
                            ident_bf[:],
                        )
                    nc.vector.tensor_copy(
                        xts[j][:, 512 * ig : 512 * (ig + 1)], pt[:, :512]
                    )
                for m in range(4):
                    i = 4 * ig + m
                    ps = psP.tile([128, 512], FP, name="psp", tag="psp")
                    for e in range(EB):
                        nc.tensor.matmul(
                            ps[:],
                            xts[e][:, 128 * i : 128 * (i + 1)],
                            wv_sb[e][:],
                            start=(e == 0), stop=(e == EB - 1),
                        )
                    nc.vector.tensor_copy(vsb[:, 512 * i : 512 * (i + 1)], ps[:])

            for h in range(H_LOC):
                for wsb, dstl, bcol in (
                    (wq_sb, qT, h),
                    (wk_sb, kT, H_LOC + h),
                ):
                    for sc in range(4):
                        ps = psP.tile([128, 512], FP, name="psp", tag="psp")
                        for e in range(EB):
                            nc.tensor.matmul(
                                ps[:],
                                wsb[e][:, 128 * h : 128 * (h + 1)],
                                xts[e][:, 512 * sc : 512 * (sc + 1)],
                                start=(e == 0), stop=(e == EB - 1),
                            )
                        nc.vector.tensor_scalar_add(
                            dstl[h][:, 512 * sc : 512 * (sc + 1)],
                            ps[:], bqk[:, bcol : bcol + 1],
                        )

        with ExitStack() as ao:
            psF = ao.enter_context(tc.tile_pool(name="psF", bufs=3, space="PSUM"))
            psT = ao.enter_context(tc.tile_pool(name="psT", bufs=2, space="PSUM"))
            att_pool = ao.enter_context(tc.tile_pool(name="att", bufs=1))
            attT = [att_pool.tile([128, S], BF, name=f"attT{h}", tag=f"attT{h}")
                    for h in range(H_LOC)]
            wo_pool = ao.enter_context(tc.tile_pool(name="wo", bufs=1))
            wo_sb = [wo_pool.tile([128, E], BF, name=f"wo{h}", tag=f"wo{h}")
                     for h in range(H_LOC)]
            for h in range(H_LOC):
                nc.scalar.dma_start(
                    wo_sb[h][:], wo_d[128 * h : 128 * (h + 1), :]
                )

            with ExitStack() as ph:
                p_pool = ph.enter_context(tc.tile_pool(name="p", bufs=14))
                pt_pool = ph.enter_context(tc.tile_pool(name="pt", bufs=2))
                rs_pool = ph.enter_context(tc.tile_pool(name="rs", bufs=10))
                PTs = {}

                pending = deque()

                def emit_some(n):
                    for _ in range(n):
                        if pending:
                            pending.popleft()()

                def sc_block(h, g, qs, pts):
                    i = 4 * g + qs
                    L = 128 * (i + 1)
                    nch = (L + 1023) // 1024
                    p_i = p_pool.tile([128, S], BF, name="p", tag="p")
                    rs = rs_pool.tile([128, 4], FP, name="rs", tag="rs")
                    pts.append(p_i)
                    qblk = qT[h][:, 128 * i : 128 * (i + 1)]
                    for c in range(nch):
                        w = min(1024, L - 1024 * c)
                        base = 1024 * c
                        ps = psF.tile([128, 1024], FP, name="psf", tag="psf")
                        if c == nch - 1:
                            pre = w - 128
                            s = 0
                            while s < pre:
                                e2 = min(s + 512, pre)
                                nc.tensor.matmul(
                                    ps[:, s:e2], qblk,
                                    kT[h][:, base + s : base + e2],
                                    start=True, stop=True,
                                )
                                s = e2
                            nc.tensor.matmul(
                                ps[:, pre:w], qblk,
                                kT[h][:, base + pre : base + w],
                                start=True, stop=False,
                            )
                            nc.tensor.matmul(
                                ps[:, pre:w], ident_bf[:], cmask[:],
                                start=False, stop=True,
                            )
                        else:
                            for hf in (0, 512):
                                nc.tensor.matmul(
                                    ps[:, hf : hf + 512], qblk,
                                    kT[h][:, base + hf : base + hf + 512],
                                    start=True, stop=True,
                                )
                        nc.scalar.activation(
                            p_i[:, base : base + w], ps[:, :w], AF.Exp,
                            accum_out=rs[:, c : c + 1],
                        )
                    if nch > 1:
                        nc.vector.tensor_add(rs[:, 0:1], rs[:, 0:1], rs[:, 1:2])
                    nc.vector.reciprocal(rs[:, 2:3], rs[:, 0:1])
                    nc.vector.tensor_scalar_mul(p_i[:, :L], p_i[:, :L], rs[:, 2:3])

                def pt_pv_pieces(h, g, pts):
                    PT = PTs[h]
                    pieces = []
                    for jp in range(2 * g + 2):
                        def tr(jp=jp):
                            pt = psT.tile([128, 1024], BF, name="pst", tag="pst")
                            for jj, base in ((2 * jp, 0), (2 * jp + 1, 512)):
                                for qs in range(max(0, jj - 4 * g), 4):
                                    nc.tensor.transpose(
                                        pt[:, base + 128 * qs : base + 128 * (qs + 1)],
                                        pts[qs][:, 128 * jj : 128 * (jj + 1)],
                                        ident_bf[:],
                                    )
                            if jp < 2 * g:
                                nc.vector.tensor_copy(
                                    PT[:, 1024 * jp : 1024 * (jp + 1)], pt[:]
                                )
                            else:
                                m0 = 2 * jp - 4 * g
                                nc.vector.tensor_copy(
                                    PT[:, 1024 * jp + 128 * m0 : 1024 * jp + 512],
                                    pt[:, 128 * m0 : 512],
                                )
                                nc.vector.tensor_copy(
                                    PT[:, 1024 * jp + 512 + 128 * (m0 + 1) :
                                       1024 * (jp + 1)],
                                    pt[:, 512 + 128 * (m0 + 1) : 1024],
                                )
                        pieces.append(tr)

                    def pv():
                        po = psF.tile([128, 1024], FP, name="psf", tag="psf")
                        nkc = 4 * g + 4
                        for j in range(nkc):
                            qlo = max(0, 128 * (j - 4 * g))
                            nc.tensor.matmul(
                                po[:, qlo:512],
                                vsb[:, 512 * j + 128 * h : 512 * j + 128 * (h + 1)],
                                PT[:, 512 * j + qlo : 512 * (j + 1)],
                                start=(j == 0), stop=(j == nkc - 1),
                            )
                        nc.vector.tensor_copy(
                            attT[h][:, 512 * g : 512 * (g + 1)], po[:, :512]
                        )
                    pieces.append(pv)
                    return pieces

                for hp in range(H_LOC // 2):
                    h0, h1 = 2 * hp, 2 * hp + 1
                    PTs[h0] = pt_pool.tile([128, 4 * S], BF, name="PT", tag="PT")
                    PTs[h1] = pt_pool.tile([128, 4 * S], BF, name="PT", tag="PT")
                    for g in range(4):
                        for h in (h0, h1):
                            with nc.named_scope(f"attn{h}"):
                                pts = []
                                for qs in range(4):
                                    sc_block(h, g, qs, pts)
                                    emit_some(2 if g >= 2 else 1)
                                pending.extend(pt_pv_pieces(h, g, pts))
                    while pending:
                        pending.popleft()()

            with ExitStack() as ph, nc.named_scope("outproj"):
                ostg = ph.enter_context(tc.tile_pool(name="ostg", bufs=4))
                for i in range(NB):
                    psums = [psF.tile([128, 1024], FP, name="psf", tag="psf")
                             for _ in range(2)]
                    for h in range(H_LOC):
                        ah = attT[h][:, 128 * i : 128 * (i + 1)]
                        first = h == 0
                        last = h == H_LOC - 1
                        for c in range(4):
                            nc.tensor.matmul(
                                psums[c // 2][:, 512 * (c % 2) : 512 * (c % 2 + 1)],
                                ah, wo_sb[h][:, 512 * c : 512 * (c + 1)],
                                start=first, stop=last,
                            )
                    for half in range(2):
                        ot = ostg.tile([128, 1024], FP, name="ostg", tag="ostg")
                        nc.vector.tensor_copy(ot[:], psums[half][:])
                        nc.sync.dma_start(
                            out[128 * i : 128 * (i + 1),
                                1024 * half : 1024 * (half + 1)],
                            ot[:],
                        )


_NC_CACHE = None


def _get_nc():
    global _NC_CACHE
    if _NC_CACHE is None:
        nc = bacc.Bacc(
            "TRN2",
            target_bir_lowering=False,
            debug=False,
            num_devices=1,
            enable_asserts=False,
        )
        _emit(nc)
        nc.compile()
        _NC_CACHE = nc
    return _NC_CACHE


def _bf(a):
    return np.ascontiguousarray(a.astype(ml_dtypes.bfloat16))


def make_in_maps(inX, W_qkv, b_qkv, W_out):
    xh = [_bf(inX[b]) for b in range(B)]
    per_hg = []
    for hg in range(HG):
        sl = slice(FLOC * hg, FLOC * (hg + 1))
        bqk = np.empty((128, 2 * H_LOC), dtype=np.float32)
        for h in range(H_LOC):
            f0 = FLOC * hg + 128 * h
            bqk[:, h] = b_qkv[f0 : f0 + 128] * SCALE
            bqk[:, H_LOC + h] = b_qkv[E + f0 : E + f0 + 128]
        per_hg.append(
            {
                "wq": _bf(W_qkv[:, 0:E][:, sl] * SCALE),
                "wk": _bf(W_qkv[:, E : 2 * E][:, sl]),
                "wv": _bf(W_qkv[:, 2 * E : 3 * E][:, sl]),
                "bqk": bqk,
                "wo": _bf(W_out[sl, :]),
            }
        )
    return [
        {"xh": xh[c // HG], **per_hg[c % HG]} for c in range(NCORES)
    ]


def kernel(inX, W_qkv, b_qkv, W_out, b_out):
    global LAST_EXEC_NS, LAST_RESULTS
    inX = np.asarray(inX, dtype=np.float32)
    W_qkv = np.asarray(W_qkv, dtype=np.float32)
    b_qkv = np.asarray(b_qkv, dtype=np.float32)
    W_out = np.asarray(W_out, dtype=np.float32)
    b_out = np.asarray(b_out, dtype=np.float32)

    nc = _get_nc()
    in_maps = make_in_maps(inX, W_qkv, b_qkv, W_out)

    kwargs = {}
    if PROFILE:
        kwargs = {"trace": True, "trace_cores": [0]}
    res = bass_utils.run_bass_kernel_spmd(
        nc, in_maps, core_ids=list(range(NCORES)), **kwargs
    )
    LAST_EXEC_NS = res.exec_time_ns
    LAST_RESULTS = res

    bias_full = (b_out + b_qkv[2 * E : 3 * E] @ W_out).astype(np.float32)
    out = np.empty((B, S, E), dtype=np.float32)
    for b in range(B):
        acc = res.results[HG * b + 0]["out"].astype(np.float64)
        for hg in range(1, HG):
            acc += res.results[HG * b + hg]["out"]
        out[b] = (acc + bias_full).astype(np.float32)
    return out


# revision 19
# speedup vs baseline: 1.1566x; 1.0076x over previous
"""Round-15: r10 + ig0/ig1 ring-split + paired X^T (8 transposes, 1024 copies)."""

from collections import deque
from contextlib import ExitStack

import ml_dtypes
import numpy as np

import concourse.bass as bass
import concourse.tile as tile
from concourse import bacc, bass_utils, mybir
from concourse.masks import make_causal_mask, make_identity

FP = mybir.dt.float32
BF = mybir.dt.bfloat16
AF = mybir.ActivationFunctionType

B = 2
S = 2048
E = 2048
H = 16
HD = 128
NCORES = 8
HG = 4
H_LOC = H // HG
FLOC = H_LOC * HD
SCALE = 1.0 / float(np.sqrt(HD))
NEG = -1.0e30

NB = S // 128
EB = E // 128

PROFILE = False
LAST_EXEC_NS = None
LAST_RESULTS = None


def _emit(nc):
    xh = nc.dram_tensor("xh", [S, E], BF, kind="ExternalInput").ap()
    wq_d = nc.dram_tensor("wq", [E, FLOC], BF, kind="ExternalInput").ap()
    wk_d = nc.dram_tensor("wk", [E, FLOC], BF, kind="ExternalInput").ap()
    wv_d = nc.dram_tensor("wv", [E, FLOC], BF, kind="ExternalInput").ap()
    bqk_d = nc.dram_tensor("bqk", [128, 2 * H_LOC], FP, kind="ExternalInput").ap()
    wo_d = nc.dram_tensor("wo", [FLOC, E], BF, kind="ExternalInput").ap()
    out = nc.dram_tensor("out", [S, E], FP, kind="ExternalOutput").ap()

    with tile.TileContext(nc) as tc, ExitStack() as top:
        cst = top.enter_context(tc.tile_pool(name="cst", bufs=1))
        ident_bf = cst.tile([128, 128], BF, name="identbf", tag="identbf")
        make_identity(nc, ident_bf[:])
        cmask = cst.tile([128, 128], BF, name="cmask", tag="cmask")
        make_causal_mask(nc, cmask[:], mask_val=NEG)
        bqk = cst.tile([128, 2 * H_LOC], FP, name="bqk", tag="bqk")
        nc.scalar.dma_start(bqk[:], bqk_d[:, :])

        qkv_out = top.enter_context(tc.tile_pool(name="qkvo", bufs=1))
        qT = [qkv_out.tile([128, S], BF, name=f"qT{h}", tag=f"qT{h}")
              for h in range(H_LOC)]
        kT = [qkv_out.tile([128, S], BF, name=f"kT{h}", tag=f"kT{h}")
              for h in range(H_LOC)]
        vsb = qkv_out.tile([128, 4 * S], BF, name="vsb", tag="vsb")

        with ExitStack() as ph, nc.named_scope("proj"):
            psT = ph.enter_context(tc.tile_pool(name="psT", bufs=2, space="PSUM"))
            psP = ph.enter_context(tc.tile_pool(name="psP", bufs=4, space="PSUM"))
            wpool = ph.enter_context(tc.tile_pool(name="w", bufs=1))
            wv_sb = [wpool.tile([128, FLOC], BF, name=f"wv{e}", tag=f"wv{e}")
                     for e in range(EB)]
            wq_sb = [wpool.tile([128, FLOC], BF, name=f"wq{e}", tag=f"wq{e}")
                     for e in range(EB)]
            wk_sb = [wpool.tile([128, FLOC], BF, name=f"wk{e}", tag=f"wk{e}")
                     for e in range(EB)]
            xt_pool = ph.enter_context(tc.tile_pool(name="xt", bufs=1))
            xts = [xt_pool.tile([128, S], BF, name=f"xt{j}", tag=f"xt{j}")
                   for j in range(EB)]
            xin = ph.enter_context(tc.tile_pool(name="xin", bufs=8))

            def load_ig(ig):
                tiles = []
                for m in range(4):
                    i = 4 * ig + m
                    xr = xin.tile([128, E], BF, name="xin", tag="xin")
                    # first two groups ride both HWDGE rings ahead of the
                    # weight stream, halving the cold-start X latency
                    eng = nc.scalar if (ig <= 1 and m % 2) else nc.sync
                    eng.dma_start(xr[:], xh[128 * i : 128 * (i + 1), :])
                    tiles.append(xr)
                return tiles

            xrows = {0: load_ig(0), 1: load_ig(1)}
            for e in range(EB):
                nc.scalar.dma_start(wv_sb[e][:], wv_d[128 * e : 128 * (e + 1), :])
            for e in range(EB):
                nc.scalar.dma_start(wq_sb[e][:], wq_d[128 * e : 128 * (e + 1), :])
                nc.scalar.dma_start(wk_sb[e][:], wk_d[128 * e : 128 * (e + 1), :])
            for pg in range(2):  # pairs of row groups: 8 token blocks each
                if pg == 0:
                    # pair-1 rows: ring-WAR delays the transfers until the
                    # pair-0 buffers free, comfortably before they're needed
                    xrows[2] = load_ig(2)
                    xrows[3] = load_ig(3)
                rows = xrows.pop(2 * pg) + xrows.pop(2 * pg + 1)
                for j in range(EB):
                    pt = psT.tile([128, 1024], BF, name="pst", tag="pst")
                    for m in range(8):
                        nc.tensor.transpose(
                            pt[:, 128 * m : 128 * (m + 1)],
                            rows[m][:, 128 * j : 128 * (j + 1)],
                            ident_bf[:],
                        )
                    nc.vector.tensor_copy(
                        xts[j][:, 1024 * pg : 1024 * (pg + 1)], pt[:]
                    )
                for m in range(8):
                    i = 8 * pg + m
                    ps = psP.tile([128, 512], FP, name="psp", tag="psp")
                    for e in range(EB):
                        nc.tensor.matmul(
                            ps[:],
                            xts[e][:, 128 * i : 128 * (i + 1)],
                            wv_sb[e][:],
                            start=(e == 0), stop=(e == EB - 1),
                        )
                    nc.vector.tensor_copy(vsb[:, 512 * i : 512 * (i + 1)], ps[:])

            for h in range(H_LOC):
                for wsb, dstl, bcol in (
                    (wq_sb, qT, h),
                    (wk_sb, kT, H_LOC + h),
                ):
                    for sc in range(4):
                        ps = psP.tile([128, 512], FP, name="psp", tag="psp")
                        for e in range(EB):
                            nc.tensor.matmul(
                                ps[:],
                                wsb[e][:, 128 * h : 128 * (h + 1)],
                                xts[e][:, 512 * sc : 512 * (sc + 1)],
                                start=(e == 0), stop=(e == EB - 1),
                            )
                        nc.vector.tensor_scalar_add(
                            dstl[h][:, 512 * sc : 512 * (sc + 1)],
                            ps[:], bqk[:, bcol : bcol + 1],
                        )

        with ExitStack() as ao:
            psF = ao.enter_context(tc.tile_pool(name="psF", bufs=3, space="PSUM"))
            psT = ao.enter_context(tc.tile_pool(name="psT", bufs=2, space="PSUM"))
            att_pool = ao.enter_context(tc.tile_pool(name="att", bufs=1))
            attT = [att_pool.tile([128, S], BF, name=f"attT{h}", tag=f"attT{h}")
                    for h in range(H_LOC)]
            wo_pool = ao.enter_context(tc.tile_pool(name="wo", bufs=1))
            wo_sb = [wo_pool.tile([128, E], BF, name=f"wo{h}", tag=f"wo{h}")
                     for h in range(H_LOC)]
            for h in range(H_LOC):
                nc.scalar.dma_start(
                    wo_sb[h][:], wo_d[128 * h : 128 * (h + 1), :]
                )

            with ExitStack() as ph:
                p_pool = ph.enter_context(tc.tile_pool(name="p", bufs=14))
                pt_pool = ph.enter_context(tc.tile_pool(name="pt", bufs=2))
                rs_pool = ph.enter_context(tc.tile_pool(name="rs", bufs=10))
                PTs = {}

                pending = deque()

                def emit_some(n):
                    for _ in range(n):
                        if pending:
                            pending.popleft()()

                def sc_block(h, g, qs, pts):
                    i = 4 * g + qs
                    L = 128 * (i + 1)
                    nch = (L + 1023) // 1024
                    p_i = p_pool.tile([128, S], BF, name="p", tag="p")
                    rs = rs_pool.tile([128, 4], FP, name="rs", tag="rs")
                    pts.append(p_i)
                    qblk = qT[h][:, 128 * i : 128 * (i + 1)]
                    for c in range(nch):
                        w = min(1024, L - 1024 * c)
                        base = 1024 * c
                        ps = psF.tile([128, 1024], FP, name="psf", tag="psf")
                        if c == nch - 1:
                            pre = w - 128
                            s = 0
                            while s < pre:
                                e2 = min(s + 512, pre)
                                nc.tensor.matmul(
                                    ps[:, s:e2], qblk,
                                    kT[h][:, base + s : base + e2],
                                    start=True, stop=True,
                                )
                                s = e2
                            nc.tensor.matmul(
                                ps[:, pre:w], qblk,
                                kT[h][:, base + pre : base + w],
                                start=True, stop=False,
                            )
                            nc.tensor.matmul(
                                ps[:, pre:w], ident_bf[:], cmask[:],
                                start=False, stop=True,
                            )
                        else:
                            for hf in (0, 512):
                                nc.tensor.matmul(
                                    ps[:, hf : hf + 512], qblk,
                                    kT[h][:, base + hf : base + hf + 512],
                                    start=True, stop=True,
                                )
                        nc.scalar.activation(
                            p_i[:, base : base + w], ps[:, :w], AF.Exp,
                            accum_out=rs[:, c : c + 1],
                        )
                    if nch > 1:
                        nc.vector.tensor_add(rs[:, 0:1], rs[:, 0:1], rs[:, 1:2])
                    nc.vector.reciprocal(rs[:, 2:3], rs[:, 0:1])
                    nc.vector.tensor_scalar_mul(p_i[:, :L], p_i[:, :L], rs[:, 2:3])

                def pt_pv_pieces(h, g, pts):
                    PT = PTs[h]
                    pieces = []
                    for jp in range(2 * g + 2):
                        def tr(jp=jp):
                            pt = psT.tile([128, 1024], BF, name="pst", tag="pst")
                            for jj, base in ((2 * jp, 0), (2 * jp + 1, 512)):
                                for qs in range(max(0, jj - 4 * g), 4):
                                    nc.tensor.transpose(
                                        pt[:, base + 128 * qs : base + 128 * (qs + 1)],
                                        pts[qs][:, 128 * jj : 128 * (jj + 1)],
                                        ident_bf[:],
                                    )
                            if jp < 2 * g:
                                nc.vector.tensor_copy(
                                    PT[:, 1024 * jp : 1024 * (jp + 1)], pt[:]
                                )
                            else:
                                m0 = 2 * jp - 4 * g
                                nc.vector.tensor_copy(
                                    PT[:, 1024 * jp + 128 * m0 : 1024 * jp + 512],
                                    pt[:, 128 * m0 : 512],
                                )
                                nc.vector.tensor_copy(
                                    PT[:, 1024 * jp + 512 + 128 * (m0 + 1) :
                                       1024 * (jp + 1)],
                                    pt[:, 512 + 128 * (m0 + 1) : 1024],
                                )
                        pieces.append(tr)

                    def pv():
                        po = psF.tile([128, 1024], FP, name="psf", tag="psf")
                        nkc = 4 * g + 4
                        for j in range(nkc):
                            qlo = max(0, 128 * (j - 4 * g))
                            nc.tensor.matmul(
                                po[:, qlo:512],
                                vsb[:, 512 * j + 128 * h : 512 * j + 128 * (h + 1)],
                                PT[:, 512 * j + qlo : 512 * (j + 1)],
                                start=(j == 0), stop=(j == nkc - 1),
                            )
                        nc.vector.tensor_copy(
                            attT[h][:, 512 * g : 512 * (g + 1)], po[:, :512]
                        )
                    pieces.append(pv)
                    return pieces

                for hp in range(H_LOC // 2):
                    h0, h1 = 2 * hp, 2 * hp + 1
                    PTs[h0] = pt_pool.tile([128, 4 * S], BF, name="PT", tag="PT")
                    PTs[h1] = pt_pool.tile([128, 4 * S], BF, name="PT", tag="PT")
                    for g in range(4):
                        for h in (h0, h1):
                            with nc.named_scope(f"attn{h}"):
                                pts = []
                                for qs in range(4):
                                    sc_block(h, g, qs, pts)
                                    emit_some(2 if g >= 2 else 1)
                                pending.extend(pt_pv_pieces(h, g, pts))
                    while pending:
                        pending.popleft()()

            with ExitStack() as ph, nc.named_scope("outproj"):
                ostg = ph.enter_context(tc.tile_pool(name="ostg", bufs=4))
                for i in range(NB):
                    psums = [psF.tile([128, 1024], FP, name="psf", tag="psf")
                             for _ in range(2)]
                    for h in range(H_LOC):
                        ah = attT[h][:, 128 * i : 128 * (i + 1)]
                        first = h == 0
                        last = h == H_LOC - 1
                        for c in range(4):
                            nc.tensor.matmul(
                                psums[c // 2][:, 512 * (c % 2) : 512 * (c % 2 + 1)],
                                ah, wo_sb[h][:, 512 * c : 512 * (c + 1)],
                                start=first, stop=last,
                            )
                    for half in range(2):
                        ot = ostg.tile([128, 1024], FP, name="ostg", tag="ostg")
                        nc.vector.tensor_copy(ot[:], psums[half][:])
                        nc.sync.dma_start(
                            out[128 * i : 128 * (i + 1),
                                1024 * half : 1024 * (half + 1)],
                            ot[:],
                        )


_NC_CACHE = None


def _get_nc():
    global _NC_CACHE
    if _NC_CACHE is None:
        nc = bacc.Bacc(
            "TRN2",
            target_bir_lowering=False,
            debug=False,
            num_devices=1,
            enable_asserts=False,
        )
        _emit(nc)
        nc.compile()
        _NC_CACHE = nc
    return _NC_CACHE


def _bf(a):
    return np.ascontiguousarray(a.astype(ml_dtypes.bfloat16))


def make_in_maps(inX, W_qkv, b_qkv, W_out):
    xh = [_bf(inX[b]) for b in range(B)]
    per_hg = []
    for hg in range(HG):
        sl = slice(FLOC * hg, FLOC * (hg + 1))
        bqk = np.empty((128, 2 * H_LOC), dtype=np.float32)
        for h in range(H_LOC):
            f0 = FLOC * hg + 128 * h
            bqk[:, h] = b_qkv[f0 : f0 + 128] * SCALE
            bqk[:, H_LOC + h] = b_qkv[E + f0 : E + f0 + 128]
        per_hg.append(
            {
                "wq": _bf(W_qkv[:, 0:E][:, sl] * SCALE),
                "wk": _bf(W_qkv[:, E : 2 * E][:, sl]),
                "wv": _bf(W_qkv[:, 2 * E : 3 * E][:, sl]),
                "bqk": bqk,
                "wo": _bf(W_out[sl, :]),
            }
        )
    return [
        {"xh": xh[c // HG], **per_hg[c % HG]} for c in range(NCORES)
    ]


def kernel(inX, W_qkv, b_qkv, W_out, b_out):
    global LAST_EXEC_NS, LAST_RESULTS
    inX = np.asarray(inX, dtype=np.float32)
    W_qkv = np.asarray(W_qkv, dtype=np.float32)
    b_qkv = np.asarray(b_qkv, dtype=np.float32)
    W_out = np.asarray(W_out, dtype=np.float32)
    b_out = np.asarray(b_out, dtype=np.float32)

    nc = _get_nc()
    in_maps = make_in_maps(inX, W_qkv, b_qkv, W_out)

    kwargs = {}
    if PROFILE:
        kwargs = {"trace": True, "trace_cores": [0]}
    res = bass_utils.run_bass_kernel_spmd(
        nc, in_maps, core_ids=list(range(NCORES)), **kwargs
    )
    LAST_EXEC_NS = res.exec_time_ns
    LAST_RESULTS = res

    bias_full = (b_out + b_qkv[2 * E : 3 * E] @ W_out).astype(np.float32)
    out = np.empty((B, S, E), dtype=np.float32)
    for b in range(B):
        acc = res.results[HG * b + 0]["out"].astype(np.float64)
        for hg in range(1, HG):
            acc += res.results[HG * b + hg]["out"]
        out[b] = (acc + bias_full).astype(np.float32)
    return out
